# revision 10
# baseline (speedup 1.0000x reference)
"""Trainium2 Bass kernel for pairwise-MLP GNN message passing.

dro[b,i,j] = W3^T relu(W2^T relu(PhiA_i + PhiB_j ... ) + b2) + b3 with the
first linear layer factorized as hA_i + hB_j (no relu between concat and W1).

Sharding: robot-row dimension N=512 split across 8 cores (64 rows each);
all other tensors replicated. Each core computes a [B, 64, N] slab.

Math rewrite used on device (host does only O(H^2) weight prep):
  dro[b,i,j] = sum_h s_h * relu(z'[j,h]) + b3
  z'[j,:]    = t1e[:,j]^T @ W2e          (PE, float32r, K=321)
  t1e[k,j]   = relu(hA[b,i,k] + hBT[b][k,j])   k<320;  t1e[320,j] = 1
  W2e        = [[W2 * |w3|][perm] ; (b2*|w3|)[perm]],  s = sign(w3)[perm]
with perm putting s>=0 columns first so the h-sum splits into two
contiguous relu+rowsum reductions (fused on ACT via accum_out), minus
variant handled by a signed multiply on DVE.
"""

import numpy as np

import concourse.bass as bass
import concourse.mybir as mybir
import concourse.tile as tile
from concourse import bacc
from concourse import bass_utils
from concourse.masks import make_identity

F32 = mybir.dt.float32
F32R = mybir.dt.float32r
ALU = mybir.AluOpType
ACTF = mybir.ActivationFunctionType

B, N, E, L = 2, 512, 128, 32
D = E + L            # 160
H = 2 * D            # 320
NCORES = 8
NI = N // NCORES     # 64 robot rows per core
KS = [(0, 128), (128, 128), (256, 65)]   # k-tiles of H+1=321 (last has ones row)
MS = [(0, 128), (128, 128), (256, 64)]   # m-tiles of H=320 (hA/hB build)
NJT = 4                                   # j-tiles of 128

# L1 runs on ACT (activation Relu with per-partition bias, SBUF->SBUF);
# all of L3 runs on DVE (scalar_tensor_tensor relu*signs with cheap
# accumulator readout - ACT's ACTIVATION_READ_ACCUMULATOR costs ~600ns vs
# DVE's 83ns, measured).
ACT_JTS = ()
DVE_JTS = (0, 1, 2, 3)

_CACHE = {}


def _build(npos: int):
    nc = bacc.Bacc("TRN2", target_bir_lowering=False, debug=False,
                   enable_asserts=False, num_devices=NCORES)

    robot = nc.dram_tensor("robot", [B, NI, E], F32, kind="ExternalInput").ap()
    obj = nc.dram_tensor("obj", [B, N, E], F32, kind="ExternalInput").ap()
    W1A = nc.dram_tensor("W1A", [E, H], F32, kind="ExternalInput").ap()
    W1B = nc.dram_tensor("W1B", [E, H], F32, kind="ExternalInput").ap()
    zAT = nc.dram_tensor("zAT", [H, B], F32, kind="ExternalInput").ap()
    zBT = nc.dram_tensor("zBT", [H, B], F32, kind="ExternalInput").ap()
    W2e = nc.dram_tensor("W2e", [H + 1, H], F32, kind="ExternalInput").ap()
    signs = nc.dram_tensor("signs", [128, H], F32, kind="ExternalInput").ap()
    b3col = nc.dram_tensor("b3col", [128, 1], F32, kind="ExternalInput").ap()
    out = nc.dram_tensor("out", [B, NI, N], F32, kind="ExternalOutput").ap()

    with tile.TileContext(nc) as tc:
        with tc.tile_pool(name="persist", bufs=1) as pp:
            # ---- persistent tiles ----
            ident = pp.tile([128, 128], F32, tag="ident")
            make_identity(nc, ident[:])
            sg = pp.tile([128, H], F32, tag="sg")
            nc.scalar.dma_start(sg[:], signs)
            # force the ACT function-table load early so it overlaps setup
            warm = pp.tile([1, 1], F32, tag="warm")
            nc.scalar.activation(warm[:], sg[0:1, 0:1], ACTF.Relu)
            b3 = pp.tile([128, 1], F32, tag="b3")
            nc.scalar.dma_start(b3[:], b3col)
            # f32r weight tiles (must be produced by a compute engine);
            # spread input DMAs across engine queues so they run in parallel
            with tc.tile_pool(name="wstg", bufs=5) as wstg:
                stg = wstg.tile([E, H], F32, tag="wstg")
                nc.sync.dma_start(stg[:], W1A)
                w1a = pp.tile([E, H], F32R, tag="w1a")
                nc.vector.tensor_copy(w1a[:], stg[:])
                stg = wstg.tile([E, H], F32, tag="wstg")
                nc.gpsimd.dma_start(stg[:], W1B)
                w1b = pp.tile([E, H], F32R, tag="w1b")
                nc.vector.tensor_copy(w1b[:], stg[:])
                w2 = []
                dmae = [nc.sync, nc.scalar, nc.gpsimd]
                for k, (k0, sz) in enumerate(KS):
                    stg = wstg.tile([sz, H], F32, tag="wstg")
                    dmae[k].dma_start(stg[:], W2e[k0:k0 + sz, :])
                    t = pp.tile([sz, H], F32R, tag=f"w2_{k}")
                    nc.vector.tensor_copy(t[:], stg[:])
                    w2.append(t)
            zat, zbt = [], []
            for m, (m0, sz) in enumerate(MS):
                t = pp.tile([sz, B], F32, tag=f"zat_{m}")
                nc.gpsimd.dma_start(t[:], zAT[m0:m0 + sz, :])
                zat.append(t)
                t = pp.tile([sz, B], F32, tag=f"zbt_{m}")
                nc.gpsimd.dma_start(t[:], zBT[m0:m0 + sz, :])
                zbt.append(t)

            hbt = {}  # (b, k) -> [szk, N] tile,  k=2 has ones row at 64
            hat = {}  # (b, k) -> [szk, NI] tile, k=2 has zeros row at 64

            # ---- setup: build hA^T, hB^T on device ----
            with tc.tile_pool(name="s_sb", bufs=2) as ssb, \
                 tc.tile_pool(name="s_ps", bufs=2, space="PSUM") as sps:
                for b in range(B):
                    # hB^T[b]: [H, N] from obj[b] @ W1B (+ zB bias)
                    objT_ps = sps.tile([128, N], F32, tag="objT_ps")
                    for jt in range(NJT):
                        stg = ssb.tile([128, E], F32, tag="stg", bufs=2)
                        [nc.sync, nc.scalar, nc.gpsimd, nc.sync][jt].dma_start(
                            stg[:], obj[b, jt * 128:(jt + 1) * 128, :])
                        nc.tensor.transpose(objT_ps[:, jt * 128:(jt + 1) * 128],
                                            stg[:], ident[:])
                    objT = ssb.tile([128, N], F32R, tag="objT")
                    nc.vector.tensor_copy(objT[:], objT_ps[:])
                    for m, (m0, sz) in enumerate(MS):
                        hps = sps.tile([sz, N], F32, tag="hps")
                        nc.tensor.matmul(hps[:], w1b[:, m0:m0 + sz], objT[:],
                                         start=True, stop=True)
                        szk = KS[m][1]
                        t = pp.tile([szk, N], F32, tag=f"hbt_{b}_{m}")
                        nc.vector.tensor_scalar(out=t[0:sz, :], in0=hps[:],
                                                scalar1=zbt[m][:, b:b + 1],
                                                scalar2=None, op0=ALU.add)
                        if m == 2:
                            nc.gpsimd.memset(t[64:65, :], 1.0)
                        hbt[(b, m)] = t

                    # hA^T[b]: [H, NI] from robot[b] @ W1A (+ zA bias)
                    stg2 = ssb.tile([NI, E], F32, tag="stg2")
                    nc.gpsimd.dma_start(stg2[:], robot[b, :, :])
                    robT_ps = sps.tile([128, NI], F32, tag="robT_ps")
                    nc.tensor.transpose(robT_ps[:], stg2[:], ident[0:NI, 0:NI])
                    robT = ssb.tile([128, NI], F32R, tag="robT")
                    nc.vector.tensor_copy(robT[:], robT_ps[:])
                    for m, (m0, sz) in enumerate(MS):
                        aps_ = sps.tile([sz, NI], F32, tag="aps")
                        nc.tensor.matmul(aps_[:], w1a[:, m0:m0 + sz], robT[:],
                                         start=True, stop=True)
                        szk = KS[m][1]
                        t = pp.tile([szk, NI], F32, tag=f"hat_{b}_{m}")
                        nc.vector.tensor_scalar(out=t[0:sz, :], in0=aps_[:],
                                                scalar1=zat[m][:, b:b + 1],
                                                scalar2=None, op0=ALU.add)
                        if m == 2:
                            nc.gpsimd.memset(t[64:65, :], 0.0)
                        hat[(b, m)] = t

            # ---- main loop ----
            with tc.tile_pool(name="t1p", bufs=3) as t1p, \
                 tc.tile_pool(name="z2p", bufs=2, space="PSUM") as z2p, \
                 tc.tile_pool(name="scr", bufs=4) as scr, \
                 tc.tile_pool(name="accp", bufs=2) as accp, \
                 tc.tile_pool(name="outp", bufs=2) as outp:
                for b in range(B):
                    opos = {jt: accp.tile([128, NI], F32, tag=f"opos_{jt}",
                                           name=f"opos_{jt}_{b}")
                            for jt in ACT_JTS}
                    oneg = {jt: accp.tile([128, NI], F32, tag=f"oneg_{jt}",
                                           name=f"oneg_{jt}_{b}")
                            for jt in ACT_JTS}
                    osig = {jt: accp.tile([128, NI], F32, tag=f"osig_{jt}",
                                           name=f"osig_{jt}_{b}")
                            for jt in DVE_JTS}

                    for i in range(NI):
                        # L1: t1_k = relu(hBT_k + hA_col); k0/k1 on ACT,
                        # k2 on the otherwise-idle GPSIMD
                        t1 = []
                        for k, (_, szk) in enumerate(KS):
                            t = t1p.tile([szk, N], F32R, tag=f"t1_{k}")
                            if k == 2:
                                nc.gpsimd.tensor_scalar(
                                    out=t[:], in0=hbt[(b, k)][:],
                                    scalar1=hat[(b, k)][:, i:i + 1],
                                    scalar2=0.0, op0=ALU.add, op1=ALU.max)
                            else:
                                nc.scalar.activation(
                                    t[:], hbt[(b, k)][:], ACTF.Relu,
                                    bias=hat[(b, k)][:, i:i + 1])
                            t1.append(t)
                        # L2: z2[jt] = t1^T @ W2e  (PE, f32r)
                        z2 = []
                        for jt in range(NJT):
                            zt = z2p.tile([128, H], F32, tag=f"z2_{jt}")
                            for k in range(3):
                                nc.tensor.matmul(
                                    zt[:], t1[k][:, jt * 128:(jt + 1) * 128],
                                    w2[k][:], start=(k == 0), stop=(k == 2))
                            z2.append(zt)
                        # L3: fused relu + row-reduce
                        for jt in ACT_JTS:
                            s = scr.tile([128, H], F32, tag=f"scr_{jt % 2}")
                            nc.scalar.activation(s[:, 0:npos], z2[jt][:, 0:npos],
                                                 ACTF.Relu,
                                                 accum_out=opos[jt][:, i:i + 1])
                            nc.scalar.activation(s[:, npos:H], z2[jt][:, npos:H],
                                                 ACTF.Relu,
                                                 accum_out=oneg[jt][:, i:i + 1])
                        for jt in DVE_JTS:
                            s = scr.tile([128, H], F32, tag="scr_d")
                            nc.vector.scalar_tensor_tensor(
                                out=s[:], in0=z2[jt][:], scalar=0.0, in1=sg[:],
                                op0=ALU.max, op1=ALU.mult,
                                accum_out=osig[jt][:, i:i + 1])

                    # epilogue for batch b: combine, transpose, store
                    osb = outp.tile([NI, N], F32, tag="osb")
                    for jt in range(NJT):
                        oc = outp.tile([128, NI], F32, tag=f"oc_{jt % 2}")
                        if jt in ACT_JTS:
                            nc.vector.scalar_tensor_tensor(
                                out=oc[:], in0=opos[jt][:], scalar=b3[:, 0:1],
                                in1=oneg[jt][:], op0=ALU.add, op1=ALU.subtract)
                        else:
                            nc.vector.tensor_scalar(
                                out=oc[:], in0=osig[jt][:], scalar1=b3[:, 0:1],
                                scalar2=None, op0=ALU.add)
                        tp = z2p.tile([NI, 128], F32, tag=f"z2_{jt}")
                        nc.tensor.transpose(tp[:], oc[:], ident[:])
                        nc.scalar.copy(osb[:, jt * 128:(jt + 1) * 128], tp[:])
                        nc.sync.dma_start(out[b, :, jt * 128:(jt + 1) * 128],
                                          osb[:, jt * 128:(jt + 1) * 128])

    nc.compile()
    return nc


def _prep(robot_embedding_tf, object_embedding_tf, z, W1, b1, W2, b2, W3, b3):
    """Host-side weight prep (O(H^2)) + per-core input maps."""
    f = np.float32
    robot = np.ascontiguousarray(robot_embedding_tf, dtype=f)
    obj = np.ascontiguousarray(object_embedding_tf, dtype=f)
    z = np.asarray(z, dtype=f)
    W1 = np.asarray(W1, dtype=f)
    b1 = np.asarray(b1, dtype=f)
    W2 = np.asarray(W2, dtype=f)
    b2 = np.asarray(b2, dtype=f)
    W3 = np.asarray(W3, dtype=f)
    b3 = np.asarray(b3, dtype=f)

    w3 = W3[:, 0]
    aw3 = np.abs(w3)
    s = np.sign(w3)
    perm = np.argsort(s < 0, kind="stable")  # s>=0 first
    npos = int(np.sum(s >= 0))
    W2p = (W2 * aw3[None, :])[:, perm]
    b2p = (b2 * aw3)[perm]
    sp = s[perm]
    W2e = np.ascontiguousarray(np.vstack([W2p, b2p[None, :]]), dtype=f)
    signs = np.ascontiguousarray(np.broadcast_to(sp[None, :], (128, H)), dtype=f)
    b3col = np.full((128, 1), b3[0], dtype=f)

    zA = z @ W1[E:D, :]                 # [B, H]
    zB = z @ W1[D + E:, :] + b1[None, :]
    zAT = np.ascontiguousarray(zA.T, dtype=f)
    zBT = np.ascontiguousarray(zB.T, dtype=f)
    W1A = np.ascontiguousarray(W1[0:E, :], dtype=f)
    W1B = np.ascontiguousarray(W1[D:D + E, :], dtype=f)

    shared = dict(obj=obj, W1A=W1A, W1B=W1B, zAT=zAT, zBT=zBT, W2e=W2e,
                  signs=signs, b3col=b3col)
    in_maps = []
    for c in range(NCORES):
        m = dict(shared)
        m["robot"] = np.ascontiguousarray(robot[:, c * NI:(c + 1) * NI, :])
        in_maps.append(m)
    return in_maps, npos


def _run(trace=False, **inputs):
    in_maps, npos = _prep(**inputs)
    key = ("nc", npos)
    if key not in _CACHE:
        _CACHE[key] = _build(npos)
    nc = _CACHE[key]
    res = bass_utils.run_bass_kernel_spmd(
        nc, in_maps, core_ids=list(range(NCORES)), trace=trace)
    dro = np.empty((B, N, N), dtype=np.float32)
    for c in range(NCORES):
        dro[:, c * NI:(c + 1) * NI, :] = res.results[c]["out"]
    return dro, res


def kernel(**inputs) -> np.ndarray:
    dro, _ = _run(trace=False, **inputs)
    return dro


# revision 11
# speedup vs baseline: 3.6143x; 3.6143x over previous
"""Trainium2 Bass kernel for pairwise-MLP GNN message passing.

dro[b,i,j] = W3^T relu(W2^T relu(PhiA_i + PhiB_j ... ) + b2) + b3 with the
first linear layer factorized as hA_i + hB_j (no relu between concat and W1).

Sharding: robot-row dimension N=512 split across 8 cores (64 rows each);
all other tensors replicated. Each core computes a [B, 64, N] slab.

Math rewrite used on device (host does only O(H^2) weight prep):
  dro[b,i,j] = sum_h s_h * relu(z'[j,h]) + b3
  z'[j,:]    = t1e[:,j]^T @ W2e          (PE, float32r, K=321)
  t1e[k,j]   = relu(hA[b,i,k] + hBT[b][k,j])   k<320;  t1e[320,j] = 1
  W2e        = [[W2 * |w3|][perm] ; (b2*|w3|)[perm]],  s = sign(w3)[perm]
with perm putting s>=0 columns first so the h-sum splits into two
contiguous relu+rowsum reductions (fused on ACT via accum_out), minus
variant handled by a signed multiply on DVE.
"""

import numpy as np

import concourse.bass as bass
import concourse.mybir as mybir
import concourse.tile as tile
from concourse import bacc
from concourse import bass_utils
from concourse.masks import make_identity

F32 = mybir.dt.float32
F32R = mybir.dt.float32r
ALU = mybir.AluOpType
ACTF = mybir.ActivationFunctionType

B, N, E, L = 2, 512, 128, 32
D = E + L            # 160
H = 2 * D            # 320
NCORES = 8
NI = N // NCORES     # 64 robot rows per core
KS = [(0, 128), (128, 128), (256, 65)]   # k-tiles of H+1=321 (last has ones row)
MS = [(0, 128), (128, 128), (256, 64)]   # m-tiles of H=320 (hA/hB build)
NJT = 4                                   # j-tiles of 128

# L1 runs on ACT (activation Relu with per-partition bias, SBUF->SBUF);
# all of L3 runs on DVE (scalar_tensor_tensor relu*signs with cheap
# accumulator readout - ACT's ACTIVATION_READ_ACCUMULATOR costs ~600ns vs
# DVE's 83ns, measured).
ACT_JTS = ()
DVE_JTS = (0, 1, 2, 3)

_CACHE = {}


def _build(npos: int):
    nc = bacc.Bacc("TRN2", target_bir_lowering=False, debug=False,
                   enable_asserts=False, num_devices=NCORES)

    robot = nc.dram_tensor("robot", [B, NI, E], F32, kind="ExternalInput").ap()
    obj = nc.dram_tensor("obj", [B, N, E], F32, kind="ExternalInput").ap()
    W1A = nc.dram_tensor("W1A", [E, H], F32, kind="ExternalInput").ap()
    W1B = nc.dram_tensor("W1B", [E, H], F32, kind="ExternalInput").ap()
    zAT = nc.dram_tensor("zAT", [H, B], F32, kind="ExternalInput").ap()
    zBT = nc.dram_tensor("zBT", [H, B], F32, kind="ExternalInput").ap()
    W2e = nc.dram_tensor("W2e", [H + 1, H], F32, kind="ExternalInput").ap()
    signs = nc.dram_tensor("signs", [128, H], F32, kind="ExternalInput").ap()
    b3col = nc.dram_tensor("b3col", [128, 1], F32, kind="ExternalInput").ap()
    out = nc.dram_tensor("out", [B, NI, N], F32, kind="ExternalOutput").ap()

    with tile.TileContext(nc) as tc:
        with tc.tile_pool(name="persist", bufs=1) as pp:
            # ---- persistent tiles ----
            ident = pp.tile([128, 128], F32, tag="ident")
            make_identity(nc, ident[:])
            sg = pp.tile([128, H], F32, tag="sg")
            nc.scalar.dma_start(sg[:], signs)
            # force the ACT function-table load early so it overlaps setup
            warm = pp.tile([1, 1], F32, tag="warm")
            nc.scalar.activation(warm[:], sg[0:1, 0:1], ACTF.Relu)
            b3 = pp.tile([128, 1], F32, tag="b3")
            nc.scalar.dma_start(b3[:], b3col)
            # f32r weight tiles (must be produced by a compute engine);
            # spread input DMAs across engine queues so they run in parallel
            with tc.tile_pool(name="wstg", bufs=5) as wstg:
                stg = wstg.tile([E, H], F32, tag="wstg")
                nc.sync.dma_start(stg[:], W1A)
                w1a = pp.tile([E, H], F32R, tag="w1a")
                nc.vector.tensor_copy(w1a[:], stg[:])
                stg = wstg.tile([E, H], F32, tag="wstg")
                nc.gpsimd.dma_start(stg[:], W1B)
                w1b = pp.tile([E, H], F32R, tag="w1b")
                nc.vector.tensor_copy(w1b[:], stg[:])
                w2 = []
                dmae = [nc.sync, nc.scalar, nc.gpsimd]
                for k, (k0, sz) in enumerate(KS):
                    stg = wstg.tile([sz, H], F32, tag="wstg")
                    dmae[k].dma_start(stg[:], W2e[k0:k0 + sz, :])
                    t = pp.tile([sz, H], F32R, tag=f"w2_{k}")
                    nc.vector.tensor_copy(t[:], stg[:])
                    w2.append(t)
            zat, zbt = [], []
            for m, (m0, sz) in enumerate(MS):
                t = pp.tile([sz, B], F32, tag=f"zat_{m}")
                nc.gpsimd.dma_start(t[:], zAT[m0:m0 + sz, :])
                zat.append(t)
                t = pp.tile([sz, B], F32, tag=f"zbt_{m}")
                nc.gpsimd.dma_start(t[:], zBT[m0:m0 + sz, :])
                zbt.append(t)

            hbt = {}  # (b, k) -> [szk, N] tile,  k=2 has ones row at 64
            hat = {}  # (b, k) -> [szk, NI] tile, k=2 has zeros row at 64

            # ---- setup: build hA^T, hB^T on device ----
            with tc.tile_pool(name="s_sb", bufs=2) as ssb, \
                 tc.tile_pool(name="s_ps", bufs=2, space="PSUM") as sps:
                for b in range(B):
                    # hB^T[b]: [H, N] from obj[b] @ W1B (+ zB bias)
                    objT_ps = sps.tile([128, N], F32, tag="objT_ps")
                    for jt in range(NJT):
                        stg = ssb.tile([128, E], F32, tag="stg", bufs=2)
                        [nc.sync, nc.scalar, nc.gpsimd, nc.sync][jt].dma_start(
                            stg[:], obj[b, jt * 128:(jt + 1) * 128, :])
                        nc.tensor.transpose(objT_ps[:, jt * 128:(jt + 1) * 128],
                                            stg[:], ident[:])
                    objT = ssb.tile([128, N], F32R, tag="objT")
                    nc.vector.tensor_copy(objT[:], objT_ps[:])
                    for m, (m0, sz) in enumerate(MS):
                        hps = sps.tile([sz, N], F32, tag="hps")
                        nc.tensor.matmul(hps[:], w1b[:, m0:m0 + sz], objT[:],
                                         start=True, stop=True)
                        szk = KS[m][1]
                        t = pp.tile([szk, N], F32, tag=f"hbt_{b}_{m}")
                        nc.vector.tensor_scalar(out=t[0:sz, :], in0=hps[:],
                                                scalar1=zbt[m][:, b:b + 1],
                                                scalar2=None, op0=ALU.add)
                        if m == 2:
                            nc.gpsimd.memset(t[64:65, :], 1.0)
                        hbt[(b, m)] = t

                    # hA^T[b]: [H, NI] from robot[b] @ W1A (+ zA bias)
                    stg2 = ssb.tile([NI, E], F32, tag="stg2")
                    nc.gpsimd.dma_start(stg2[:], robot[b, :, :])
                    robT_ps = sps.tile([128, NI], F32, tag="robT_ps")
                    nc.tensor.transpose(robT_ps[:], stg2[:], ident[0:NI, 0:NI])
                    robT = ssb.tile([128, NI], F32R, tag="robT")
                    nc.vector.tensor_copy(robT[:], robT_ps[:])
                    for m, (m0, sz) in enumerate(MS):
                        aps_ = sps.tile([sz, NI], F32, tag="aps")
                        nc.tensor.matmul(aps_[:], w1a[:, m0:m0 + sz], robT[:],
                                         start=True, stop=True)
                        szk = KS[m][1]
                        t = pp.tile([szk, NI], F32, tag=f"hat_{b}_{m}")
                        nc.vector.tensor_scalar(out=t[0:sz, :], in0=aps_[:],
                                                scalar1=zat[m][:, b:b + 1],
                                                scalar2=None, op0=ALU.add)
                        if m == 2:
                            nc.gpsimd.memset(t[64:65, :], 0.0)
                        hat[(b, m)] = t

            # ---- main loop ----
            with tc.tile_pool(name="t1p", bufs=3) as t1p, \
                 tc.tile_pool(name="z2p", bufs=2, space="PSUM") as z2p, \
                 tc.tile_pool(name="scr", bufs=4) as scr, \
                 tc.tile_pool(name="accp", bufs=2) as accp, \
                 tc.tile_pool(name="outp", bufs=2) as outp:
                for b in range(B):
                    opos = {jt: accp.tile([128, NI], F32, tag=f"opos_{jt}",
                                           name=f"opos_{jt}_{b}")
                            for jt in ACT_JTS}
                    oneg = {jt: accp.tile([128, NI], F32, tag=f"oneg_{jt}",
                                           name=f"oneg_{jt}_{b}")
                            for jt in ACT_JTS}
                    osig = {jt: accp.tile([128, NI], F32, tag=f"osig_{jt}",
                                           name=f"osig_{jt}_{b}")
                            for jt in DVE_JTS}

                    for i in range(NI):
                        # L1: t1_k = relu(hBT_k + hA_col) on ACT
                        # (GPSIMD tensor_scalar measured 7.5us/op - unusable)
                        t1 = []
                        for k, (_, szk) in enumerate(KS):
                            t = t1p.tile([szk, N], F32R, tag=f"t1_{k}")
                            nc.scalar.activation(
                                t[:], hbt[(b, k)][:], ACTF.Relu,
                                bias=hat[(b, k)][:, i:i + 1])
                            t1.append(t)
                        # L2: z2[jt] = t1^T @ W2e  (PE, f32r)
                        z2 = []
                        for jt in range(NJT):
                            zt = z2p.tile([128, H], F32, tag=f"z2_{jt}")
                            for k in range(3):
                                nc.tensor.matmul(
                                    zt[:], t1[k][:, jt * 128:(jt + 1) * 128],
                                    w2[k][:], start=(k == 0), stop=(k == 2))
                            z2.append(zt)
                        # L3: fused relu + row-reduce
                        for jt in ACT_JTS:
                            s = scr.tile([128, H], F32, tag=f"scr_{jt % 2}")
                            nc.scalar.activation(s[:, 0:npos], z2[jt][:, 0:npos],
                                                 ACTF.Relu,
                                                 accum_out=opos[jt][:, i:i + 1])
                            nc.scalar.activation(s[:, npos:H], z2[jt][:, npos:H],
                                                 ACTF.Relu,
                                                 accum_out=oneg[jt][:, i:i + 1])
                        for jt in DVE_JTS:
                            s = scr.tile([128, H], F32, tag="scr_d")
                            nc.vector.scalar_tensor_tensor(
                                out=s[:], in0=z2[jt][:], scalar=0.0, in1=sg[:],
                                op0=ALU.max, op1=ALU.mult,
                                accum_out=osig[jt][:, i:i + 1])

                    # epilogue for batch b: combine, transpose, store
                    osb = outp.tile([NI, N], F32, tag="osb")
                    for jt in range(NJT):
                        oc = outp.tile([128, NI], F32, tag=f"oc_{jt % 2}")
                        if jt in ACT_JTS:
                            nc.vector.scalar_tensor_tensor(
                                out=oc[:], in0=opos[jt][:], scalar=b3[:, 0:1],
                                in1=oneg[jt][:], op0=ALU.add, op1=ALU.subtract)
                        else:
                            nc.vector.tensor_scalar(
                                out=oc[:], in0=osig[jt][:], scalar1=b3[:, 0:1],
                                scalar2=None, op0=ALU.add)
                        tp = z2p.tile([NI, 128], F32, tag=f"z2_{jt}")
                        nc.tensor.transpose(tp[:], oc[:], ident[:])
                        nc.scalar.copy(osb[:, jt * 128:(jt + 1) * 128], tp[:])
                        nc.sync.dma_start(out[b, :, jt * 128:(jt + 1) * 128],
                                          osb[:, jt * 128:(jt + 1) * 128])

    nc.compile()
    return nc


def _prep(robot_embedding_tf, object_embedding_tf, z, W1, b1, W2, b2, W3, b3):
    """Host-side weight prep (O(H^2)) + per-core input maps."""
    f = np.float32
    robot = np.ascontiguousarray(robot_embedding_tf, dtype=f)
    obj = np.ascontiguousarray(object_embedding_tf, dtype=f)
    z = np.asarray(z, dtype=f)
    W1 = np.asarray(W1, dtype=f)
    b1 = np.asarray(b1, dtype=f)
    W2 = np.asarray(W2, dtype=f)
    b2 = np.asarray(b2, dtype=f)
    W3 = np.asarray(W3, dtype=f)
    b3 = np.asarray(b3, dtype=f)

    w3 = W3[:, 0]
    aw3 = np.abs(w3)
    s = np.sign(w3)
    perm = np.argsort(s < 0, kind="stable")  # s>=0 first
    npos = int(np.sum(s >= 0))
    W2p = (W2 * aw3[None, :])[:, perm]
    b2p = (b2 * aw3)[perm]
    sp = s[perm]
    W2e = np.ascontiguousarray(np.vstack([W2p, b2p[None, :]]), dtype=f)
    signs = np.ascontiguousarray(np.broadcast_to(sp[None, :], (128, H)), dtype=f)
    b3col = np.full((128, 1), b3[0], dtype=f)

    zA = z @ W1[E:D, :]                 # [B, H]
    zB = z @ W1[D + E:, :] + b1[None, :]
    zAT = np.ascontiguousarray(zA.T, dtype=f)
    zBT = np.ascontiguousarray(zB.T, dtype=f)
    W1A = np.ascontiguousarray(W1[0:E, :], dtype=f)
    W1B = np.ascontiguousarray(W1[D:D + E, :], dtype=f)

    shared = dict(obj=obj, W1A=W1A, W1B=W1B, zAT=zAT, zBT=zBT, W2e=W2e,
                  signs=signs, b3col=b3col)
    in_maps = []
    for c in range(NCORES):
        m = dict(shared)
        m["robot"] = np.ascontiguousarray(robot[:, c * NI:(c + 1) * NI, :])
        in_maps.append(m)
    return in_maps, npos


def _run(trace=False, **inputs):
    in_maps, npos = _prep(**inputs)
    key = ("nc", npos)
    if key not in _CACHE:
        _CACHE[key] = _build(npos)
    nc = _CACHE[key]
    res = bass_utils.run_bass_kernel_spmd(
        nc, in_maps, core_ids=list(range(NCORES)), trace=trace)
    dro = np.empty((B, N, N), dtype=np.float32)
    for c in range(NCORES):
        dro[:, c * NI:(c + 1) * NI, :] = res.results[c]["out"]
    return dro, res


def kernel(**inputs) -> np.ndarray:
    dro, _ = _run(trace=False, **inputs)
    return dro
